# revision 34
# baseline (speedup 1.0000x reference)
"""Causal multi-head attention (B=4, T=2048, C=1024, H=16, HD=64) on 8 TRN2
NeuronCores.

Sharding: 2D - batch (4) x head-group (2 groups of 8 heads). Each core handles
one batch's tokens for 8 heads:
    core = b * 2 + g
    host packs x (bf16 + fp8e4m3), Wq/Wk (fp8), Wv/Wo (bf16) chunk-major on
    128 partitions so every tensor/block loads with 1-8 large DMAs
    yT  [C, T]  partial: y[b] = sum_g yT_g.T    (host-side reduce over g)

Key implementation points (measured on HW):
  - Q/K projections run fp8e4m3 with MatmulPerfMode.DoubleRow: each matmul
    contracts TWO 128-chunks (lhsT [128,2,128], rhs [128,2,512]), halving the
    projection instruction count.  fp8 on the QK path only costs 9.9e-3 rel
    err (vs 2e-2 budget); fp8 anywhere on the V/O path fails (~3e-2).
  - fp8 does NOT speed up streaming: every N=512 matmul is ~259ns regardless
    of dtype/perf-mode (DoubleRow/DoublePixel measured).  The S-matmul
    (contraction 64) therefore stays bf16 with the two-heads-per-tile
    zero-padded layout.
  - V in [token, channel] layout with an all-ones column per head so the ctx
    matmul (M=65) yields the softmax denominator in psum row 64 for free.
    All ones-columns are memset ONCE upfront (a deferred memset creates a
    false WAW dep on the shared vall tile that stalls attend(1) ~6us).
  - exp via ACT (scale fused); causal masks are DVE tensor_mul with a static
    127-col triangle (only cols 0..126 of a diagonal chunk have masked rows).
    Masks on gpsimd deadlock-cycle with the norm partition_broadcast through
    the in-order queues (TM->gpsimd->scalar->TM, 6-8us stalls).
  - ctx matmuls all flush at head end (ptpool bufs=9) so a diagonal pair's
    ctx never head-of-line-blocks the TM queue on a backlogged exp.
  - Norm: denom row -> copy -> reciprocal (DVE) -> partition_broadcast
    (gpsimd) -> one DVE mul.  Deferred one head (popped after the next
    head's masks) to keep the gpsimd wait off the critical path.
  - interleave: output(tb-1) groups BEFORE project(tb+1) in the filler queue
    (output has no dependence on the in-flight x DMAs).
  - last block: output groups co0/co1 pre-open with ci0..2 before the final
    norm flush (PE keeps busy through the flush); ysb copies/DMAs split
    across scalar+vector / sync+scalar queues; ypool bufs=4 so copies don't
    wait on 2-back DMA completions.
  - PE p-state: any >3us TM idle drops the PE to half speed for ~3us
    (matmuls 427ns instead of 216ns) - gaps are doubly expensive.
"""

import numpy as np

B, T_FULL, C = 4, 2048, 1024
H, HD = 16, 64
GROUPS = 2
HL = H // GROUPS          # heads per core = 8
OC = HL * HD              # local channels = 512
P = 128                   # partitions
TB = 512                  # token block (moving dim)
VE = 65                   # V cols per head: 64 values + ones column
SCALE = float(1.0 / np.sqrt(HD))
NCORES = 8

# debug toggles (bisect); all-True is the fast path
# QUAD_S works on HW but the quadrant (K=64, tile_position) matmuls average
# ~266ns vs 247ns for full 128x128 — the padded-qt layout is net faster.
QUAD_S = False
M65 = True        # M=65 ctx lhsT (else M=128 with zeroed V pad cols)
# mask via DVE mul congests the Vector queue, which sits on the
# exp->mask->ctx critical chain; gpsimd affine_select keeps DVE clear
MASK_MUL = True
# ones-matmul denominator broadcast into PSUM rows 64:128 — BROKEN ON HW:
# a start=True matmul at column tile_position 64 clobbers the ctx rows of the
# same bank (sim models it fine; walrus/HW does not). Keep partition_broadcast.
BCAST_MM = False


def build_program(T=T_FULL):
    from contextlib import ExitStack

    import concourse.bacc as bacc
    import concourse.mybir as mybir
    import concourse.tile as tile

    f32 = mybir.dt.float32
    bf16 = mybir.dt.bfloat16
    u16 = mybir.dt.uint16
    EXP = mybir.ActivationFunctionType.Exp
    GE = mybir.AluOpType.is_ge

    NTB = T // TB             # 512-token blocks
    NKC = T // P              # 128-token key chunks
    CCH = C // P              # 8 contraction chunks of C
    MCH = OC // P             # 4 output-channel chunks

    nc = bacc.Bacc("TRN2", target_bir_lowering=False, debug=False)
    f8 = mybir.dt.float8e4
    DR = mybir.MatmulPerfMode.DoubleRow
    # host pre-packs everything chunk-major on 128 partitions so each
    # tensor (or block) is ONE big DMA instead of 8 serialized chunk DMAs
    xB = nc.dram_tensor("xB", [P, NTB * CCH * TB], bf16,
                        kind="ExternalInput").ap()
    xB8 = nc.dram_tensor("xB8", [P, NTB * CCH * TB], f8,
                         kind="ExternalInput").ap()
    wqB8 = nc.dram_tensor("wqB8", [P, CCH * OC], f8, kind="ExternalInput").ap()
    wkB8 = nc.dram_tensor("wkB8", [P, CCH * OC], f8, kind="ExternalInput").ap()
    wvB = nc.dram_tensor("wvB", [P, CCH * OC], bf16, kind="ExternalInput").ap()
    woB = nc.dram_tensor("woB", [P, MCH * C], bf16, kind="ExternalInput").ap()
    yT = nc.dram_tensor("yT", [C, T], bf16, kind="ExternalOutput").ap()

    ONE_BF16 = 0x3F80  # 1.0 in bf16 — bf16 memset via uint16 bitcast

    ve = VE if M65 else P

    with tile.TileContext(nc) as tc, ExitStack() as ctx:
        perm = ctx.enter_context(tc.tile_pool(name="perm", bufs=1))
        # two heads per tile: head 2m in rows 0:64, head 2m+1 in rows 64:128
        if QUAD_S:
            qt = [perm.tile([P, T], bf16, tag=f"qt{m}", name=f"qt{m}")
                  for m in range(MCH)]
        else:
            qt = [perm.tile([P, T], bf16, tag=f"qt{h}", name=f"qt{h}")
                  for h in range(HL)]
            # per-partition 1/0 masks: the Q-proj copy multiplies by these to
            # zero the other head's half in the same DVE op (no big memsets)
            zmask = [perm.tile([P, 1], f32, tag=f"zm{i}", name=f"zm{i}")
                     for i in range(2)]
            nc.gpsimd.memset(zmask[0][0:64, :], 1.0)
            nc.gpsimd.memset(zmask[0][64:128, :], 0.0)
            nc.gpsimd.memset(zmask[1][0:64, :], 0.0)
            nc.gpsimd.memset(zmask[1][64:128, :], 1.0)
        kt = [perm.tile([P, T], bf16, tag=f"kt{m}", name=f"kt{m}") for m in range(MCH)]
        ct = [perm.tile([P, T], bf16, tag=f"ct{m}", name=f"ct{m}") for m in range(MCH)]
        # V: per head 65 cols: [V_h | 1] so the ctx matmul (M=65) also yields
        # the softmax denominator in psum row 64.  One big tile so the ones
        # columns come from a single strided memset (32 small gpsimd memsets
        # serialized ~18us ahead of attend(0)'s masks on the gpsimd queue).
        vall = perm.tile([P, NKC * HL * ve], bf16, tag="vall", name="vall")
        v = [vall[:, t * HL * ve:(t + 1) * HL * ve] for t in range(NKC)]
        vv = vall.rearrange("p (t h e) -> p t h e", h=HL, e=ve)
        # one strided memset for ALL ones-columns upfront: a deferred memset
        # mid-pipeline creates a false WAW dep on the shared vall tile that
        # stalls attend(1)'s ctx ldweights ~6us (tile-granular dep tracking)
        nc.gpsimd.memset(vv[:, :, :, 64:65].bitcast(u16), ONE_BF16)
        if not M65:
            nc.gpsimd.memset(vv[:, :, :, 65:].bitcast(u16), 0)
        if BCAST_MM:
            # ones row for the denominator broadcast matmul (K=1, M=64)
            ones64 = perm.tile([1, 64], bf16, tag="ones64", name="ones64")
            nc.gpsimd.memset(ones64.bitcast(u16), ONE_BF16)
        if MASK_MUL:
            # one static triangle mask serves every diagonal chunk: a chunk at
            # diagonal offset d covers queries d..511 of the block, for which
            # the causal predicate is  f' >= p  — so chunk masks are prefix
            # slices mask_t[:, 0:512-d]
            mask_t = perm.tile([P, TB], bf16, tag="maskt", name="maskt")
            nc.gpsimd.memset(mask_t.bitcast(u16), ONE_BF16)
            nc.gpsimd.affine_select(
                out=mask_t, in_=mask_t, compare_op=GE, fill=0.0,
                base=0, pattern=[[1, TB]], channel_multiplier=-1)

        # ---- Fused pipeline: project(tb) -> output(tb-1) -> attend(tb) ----
        # All pools coexist; PSUM budget (8 banks): mm512 2 + st 2x2 + ctx 2.
        with (
            tc.tile_pool(name="wpool", bufs=1) as wp,
            tc.tile_pool(name="xpool", bufs=2) as xp,
            tc.tile_pool(name="ptpool", bufs=9) as ptp,
            tc.tile_pool(name="tmppool", bufs=3) as tmp,
            tc.tile_pool(name="ypool", bufs=4) as yp,
            tc.tile_pool(name="mmps", bufs=2, space="PSUM") as pp,
            tc.tile_pool(name="stps", bufs=2, space="PSUM") as stp,
            tc.tile_pool(name="ctxps", bufs=2, space="PSUM") as cxp,
        ):
            # per-chunk 2D DMAs: contiguous 1KB partition lines.  fp8 x/wq/wk
            # feed DoubleRow projection matmuls (2 contraction chunks per
            # instruction).  x-block loads ride the sync queue; weights ride
            # the scalar queue so the startup DMAs run on two queues.
            BLK = CCH * TB

            def load_x(tb):
                x8t = xp.tile([P, BLK], f8, tag="x8", name=f"x8_{tb}")
                # quad-granularity DMAs (256KB): queue occupancy is ~650ns
                # for 64KB AND 128KB (fixed overhead dominates), so bigger
                # is strictly better while still below the async threshold
                for c in range(0, CCH, 4):
                    nc.sync.dma_start(
                        out=x8t[:, c * TB:(c + 4) * TB],
                        in_=xB8[:, tb * BLK + c * TB:tb * BLK + (c + 4) * TB])
                xall = xp.tile([P, BLK], bf16, tag="xall", name=f"x_{tb}")
                nc.sync.dma_start(
                    out=xall, in_=xB[:, tb * BLK:(tb + 1) * BLK])
                xc = [xall[:, c * TB:(c + 1) * TB] for c in range(CCH)]
                return xc, x8t.rearrange("p (c n) -> p c n", n=TB)

            wq8 = wp.tile([P, CCH * OC], f8, tag="wq8", name="wq8")
            wk8 = wp.tile([P, CCH * OC], f8, tag="wk8", name="wk8")
            wvall = wp.tile([P, CCH * OC], bf16, tag="wvall", name="wvall")
            for c in range(0, CCH, 4):
                nc.scalar.dma_start(
                    out=wq8[:, c * OC:(c + 4) * OC],
                    in_=wqB8[:, c * OC:(c + 4) * OC])
            HV = CCH * OC // 2
            nc.scalar.dma_start(out=wk8[:, 0:HV], in_=wkB8[:, 0:HV])
            nc.scalar.dma_start(out=wvall[:, 0:HV], in_=wvB[:, 0:HV])
            nc.scalar.dma_start(out=wk8[:, HV:], in_=wkB8[:, HV:])
            nc.scalar.dma_start(out=wvall[:, HV:], in_=wvB[:, HV:])
            wq = wq8.rearrange("p (c o) -> p c o", o=OC)
            wk = wk8.rearrange("p (c o) -> p c o", o=OC)
            x_next, x8_next = load_x(0)
            wv = [wvall[:, c * OC:(c + 1) * OC] for c in range(CCH)]
            woall = wp.tile([P, MCH * C], bf16, tag="woall", name="woall")
            nc.sync.dma_start(out=woall, in_=woB)
            wo = [woall[:, ci * C:(ci + 1) * C] for ci in range(MCH)]

            def project_groups(tb, xc, x8, qk_interleave=False):
                groups = []

                def proj_qk(w, isq, m, tb=tb, x8=x8):
                    def go():
                        ps = pp.tile([P, TB], f32, tag="mm512",
                                     name=f"ps_{tb}_{m}_{isq}")
                        for pr in range(CCH // 2):
                            c = 2 * pr
                            nc.tensor.matmul(
                                ps, lhsT=w[:, c:c + 2, m * P:(m + 1) * P],
                                rhs=x8[:, c:c + 2, :], perf_mode=DR,
                                start=(pr == 0), stop=(pr == CCH // 2 - 1))
                        if isq and not QUAD_S:
                            # copy + zero-other-head in one DVE op each
                            for hh in (0, 1):
                                nc.vector.tensor_scalar_mul(
                                    qt[2 * m + hh][:, tb * TB:(tb + 1) * TB],
                                    ps, zmask[hh])
                        else:
                            dst = qt if isq else kt
                            nc.vector.tensor_copy(
                                dst[m][:, tb * TB:(tb + 1) * TB], ps)
                    return go

                def proj_v(ts_, tb=tb, xc=xc):
                    def go():
                        ps = pp.tile([P, OC], f32, tag="mm512",
                                     name=f"psv_{tb}_{ts_}")
                        for c in range(CCH):
                            nc.tensor.matmul(
                                ps, lhsT=xc[c][:, ts_ * P:(ts_ + 1) * P], rhs=wv[c],
                                start=(c == 0), stop=(c == CCH - 1))
                        ti = tb * (TB // P) + ts_
                        nc.vector.tensor_copy(
                            v[ti].rearrange("p (h e) -> p h e", e=ve)[:, :, 0:64],
                            ps.rearrange("p (h d) -> p h d", d=64))
                    return go

                if qk_interleave:
                    # block 0 only: attend(0) h0 needs BOTH qt[0] and kt[0]
                    # copies; interleaving puts the K m0 copy 2nd in the DVE
                    # queue instead of 9th (~3us earlier attend start)
                    for m in range(MCH):
                        groups.append(proj_qk(wq, True, m))
                        groups.append(proj_qk(wk, False, m))
                else:
                    for w, isq in ((wq, True), (wk, False)):
                        for m in range(MCH):
                            groups.append(proj_qk(w, isq, m))
                for ts_ in range(TB // P):
                    groups.append(proj_v(ts_))
                return groups

            def output_groups(tb, part=None):
                last = tb == NTB - 1

                def out_co(co, tb=tb, alloc=None):
                    def go(phase=2, ps_=[None]):
                        # phase 0: open group with ci 0..2 (before the final
                        # norm); phase 1: ci3 + copy + dma; phase 2: all
                        if phase in (0, 2):
                            if alloc is None:
                                ps_[0] = pp.tile([P, TB], f32, tag="mm512",
                                                 name=f"yps_{co}_{tb}")
                            else:
                                ps_[0] = alloc(co)
                            for ci in range(MCH - 1):
                                nc.tensor.matmul(
                                    ps_[0], lhsT=wo[ci][:, co * P:(co + 1) * P],
                                    rhs=ct[ci][:, tb * TB:(tb + 1) * TB],
                                    start=(ci == 0), stop=False)
                            if phase == 0:
                                return
                        ps = ps_[0]
                        for ci in (MCH - 1,):
                            nc.tensor.matmul(
                                ps, lhsT=wo[ci][:, co * P:(co + 1) * P],
                                rhs=ct[ci][:, tb * TB:(tb + 1) * TB],
                                start=False, stop=True)
                        ysb = yp.tile([P, TB], bf16, tag="ysb",
                                      name=f"ysb_{co}_{tb}")
                        # copies can't go to gpsimd (no PSUM access); on the
                        # final block alternate Scalar/Vector so the drain
                        # chain after the last matmul is half as long, and
                        # spread the final DMAs over both hwdge queues
                        if last and co >= 6:
                            HB = TB // 2
                            nc.scalar.copy(ysb[:, 0:HB], ps[:, 0:HB])
                            nc.vector.tensor_copy(ysb[:, HB:TB], ps[:, HB:TB])
                            nc.sync.dma_start(
                                out=yT[co * P:(co + 1) * P,
                                       tb * TB:tb * TB + HB],
                                in_=ysb[:, 0:HB])
                            nc.scalar.dma_start(
                                out=yT[co * P:(co + 1) * P,
                                       tb * TB + HB:(tb + 1) * TB],
                                in_=ysb[:, HB:TB])
                        elif last:
                            # single-engine copies alternate so each group's
                            # WAR releases after ONE copy, not two queues
                            if co % 2 == 0:
                                nc.scalar.copy(ysb, ps)
                            else:
                                nc.vector.tensor_copy(ysb, ps)
                            dq = nc.sync if co % 2 == 0 else nc.scalar
                            dq.dma_start(
                                out=yT[co * P:(co + 1) * P,
                                       tb * TB:(tb + 1) * TB],
                                in_=ysb)
                        else:
                            nc.vector.tensor_copy(ysb, ps)
                            nc.sync.dma_start(
                                out=yT[co * P:(co + 1) * P,
                                       tb * TB:(tb + 1) * TB],
                                in_=ysb)
                    return go
                if part is None:
                    return [out_co(co) for co in range(C // P)]
                return [out_co(co, alloc=part.get(co)) for co in range(C // P)]

            def output_last_split():
                # co2/co3 borrow the (idle-by-then) st psum banks, co4 the
                # free ctx buffer: 5 output groups pre-open with ci0..2
                # before the final norm, covering the norm-chain hole
                def st_alloc(co):
                    return stp.tile([P, 2 * TB], f32, tag="st",
                                    name=f"yst_{co}")[:, 0:TB]

                def cx_alloc(co):
                    return cxp.tile([P, TB], f32, tag="ctx", name=f"ycx_{co}")

                gs = output_groups(NTB - 1, part={2: st_alloc, 3: st_alloc})
                NPRE = 4
                pre = [(lambda g: (lambda: g(phase=0)))(g) for g in gs[:NPRE]]
                post = ([(lambda g: (lambda: g(phase=1)))(g) for g in gs[:NPRE]]
                        + gs[NPRE:])
                return pre, post

            pending = []

            def mk_norm(h, j, m, r0, ctx_ps, split=False):
                def norm():
                    if split:
                        # final-flush norm: pipeline the chain in two column
                        # halves so copy/recip/pbcast/mul overlap across
                        # engines instead of serializing ~3.9us.  A scratch
                        # f32 matmul dep-chained on each half's denom copy
                        # runs INSIDE the hole and keeps the PE p-state warm
                        # (post-hole matmuls otherwise run 427-585ns for ~3us)
                        dps = cxp.tile([P, TB], f32, tag="ctx", name="warmps")
                        HB2 = TB // 2
                        for hh in range(2):
                            sl = slice(hh * HB2, (hh + 1) * HB2)
                            s_sb = tmp.tile([1, TB], f32, tag="s",
                                            name=f"s_{h}_{j}_{hh}")
                            nc.vector.tensor_copy(
                                s_sb[:, 0:HB2], ctx_ps[64:65, sl])
                            r1 = tmp.tile([1, TB], f32, tag="r1",
                                          name=f"r1_{h}_{j}_{hh}")
                            nc.tensor.matmul(
                                dps, lhsT=s_sb[0:1, 0:128],
                                rhs=s_sb[0:1, 0:TB], start=True, stop=True,
                                skip_group_check=True)
                            nc.vector.reciprocal_approx_fast(
                                out=r1[:, 0:HB2], in_=s_sb[:, 0:HB2])
                            rb = tmp.tile([64, TB], f32, tag="rb",
                                          name=f"rb_{h}_{j}_{hh}")
                            nc.gpsimd.partition_broadcast(
                                rb[:, 0:HB2], r1[:, 0:HB2])
                            nc.vector.tensor_mul(
                                ct[m][r0:r0 + 64,
                                      j * TB + hh * HB2:j * TB + (hh + 1) * HB2],
                                ctx_ps[0:64, sl], rb[:, 0:HB2])
                        return
                    if BCAST_MM:
                        # denominator row -> bf16 -> broadcast into psum rows
                        # 64:128 of the same bank via ones-vector matmul
                        s_sb = tmp.tile([1, TB], bf16, tag="s", name=f"s_{h}_{j}")
                        nc.vector.tensor_copy(s_sb, ctx_ps[64:65, :])
                        nc.tensor.matmul(
                            ctx_ps[64:128, :], lhsT=ones64, rhs=s_sb,
                            start=True, stop=True, skip_group_check=True)
                        rb = tmp.tile([64, TB], f32, tag="rb", name=f"rb_{h}_{j}")
                        nc.vector.reciprocal_approx_fast(
                            out=rb, in_=ctx_ps[64:128, :])
                    else:
                        # reciprocal on the single row BEFORE the broadcast:
                        # 64x less DVE recip work.  (recip can't read PSUM
                        # directly on HW — copy the row to SBUF first.)
                        s_sb = tmp.tile([1, TB], f32, tag="s", name=f"s_{h}_{j}")
                        nc.vector.tensor_copy(s_sb, ctx_ps[64:65, :])
                        r1 = tmp.tile([1, TB], f32, tag="r1", name=f"r1_{h}_{j}")
                        nc.vector.reciprocal_approx_fast(out=r1, in_=s_sb)
                        rb = tmp.tile([64, TB], f32, tag="rb", name=f"rb_{h}_{j}")
                        nc.gpsimd.partition_broadcast(rb, r1)
                    nc.vector.tensor_mul(
                        ct[m][r0:r0 + 64, j * TB:(j + 1) * TB], ctx_ps[0:64, :], rb)
                return norm

            def attend(j, ilq, late=(), pre_flush=()):
                reserve = ilq[-2:]
                main = ilq[:max(0, len(ilq) - 2)]
                for h in range(HL):
                    if h >= 1:
                        # spread the interleave queue evenly over heads 1..7
                        npop = -(-len(main) // (HL - h))
                        for _ in range(npop):
                            if main:
                                main.pop(0)()
                    if h == HL - 1:
                        for g in late:
                            g()
                    m, r0 = h // 2, (h % 2) * 64
                    nch = 4 * (j + 1)
                    ctx_ps = cxp.tile([P, TB], f32, tag="ctx", name=f"cps_{h}_{j}")
                    npair = nch // 2
                    # pair order: wide diagonal (2j) first so its ctx matmul
                    # opens the psum group full-width with start=True, then
                    # the narrow diagonal (2j+1), then the unmasked rest.
                    # Diagonal-first keeps the long exp->mask chains early.
                    order = [2 * j, 2 * j + 1] + list(range(2 * j - 1, -1, -1))
                    inflight = []
                    nmm = [0]

                    def ctx_mms(pt_, chs, ctx_ps=ctx_ps, h=h, nch=nch):
                        for c, off, wc in chs:
                            vh = v[c].rearrange("p (h e) -> p h e", e=ve)[:, h, :]
                            nc.tensor.matmul(
                                ctx_ps[0:ve, TB - wc:TB], lhsT=vh,
                                rhs=pt_[:, off:off + wc],
                                start=(nmm[0] == 0), stop=(nmm[0] == nch - 1),
                                skip_group_check=True)
                            nmm[0] += 1

                    for idx, pp_ in enumerate(order):
                        # a diagonal chunk at offset d only matters for the
                        # last 512-d queries of the block: trim everything
                        chs = []
                        off = 0
                        for t in (0, 1):
                            c = 2 * pp_ + t
                            d = c * P - j * TB
                            wc = TB - d if d > 0 else TB
                            chs.append((c, off, wc))
                            off += wc
                        st = stp.tile([P, 2 * TB], f32, tag="st",
                                      name=f"st_{h}_{j}_{pp_}")
                        for c, off, wc in chs:
                            if QUAD_S:
                                qs = qt[m][r0:r0 + 64,
                                           (j + 1) * TB - wc:(j + 1) * TB]
                            else:
                                qs = qt[h][:, (j + 1) * TB - wc:(j + 1) * TB]
                            klhs = (kt[m][r0:r0 + 64, c * P:(c + 1) * P]
                                    if QUAD_S else kt[m][:, c * P:(c + 1) * P])
                            nc.tensor.matmul(
                                st[:, off:off + wc], lhsT=klhs, rhs=qs,
                                start=True, stop=True, skip_group_check=True)
                        pt_ = ptp.tile([P, 2 * TB], bf16, tag="pt",
                                       name=f"pt_{h}_{j}_{pp_}")
                        # one exp per pair over the packed live region (per-op
                        # ACT overhead outweighs the finer-pipelining win of
                        # per-chunk exps — measured), mask per diagonal chunk
                        wtot = chs[-1][1] + chs[-1][2]
                        nc.scalar.activation(pt_[:, 0:wtot], st[:, 0:wtot],
                                             EXP, scale=SCALE)
                        for c, off, wc in chs:
                            if c * P >= j * TB:
                                # only cols 0..126 of a diagonal chunk have
                                # any masked rows (row > col); col>=127 is
                                # fully valid, so select on 127 cols not wc
                                wm = min(wc, P - 1)
                                sl = slice(off, off + wm)
                                if MASK_MUL:
                                    nc.vector.tensor_mul(
                                        pt_[:, sl], pt_[:, sl],
                                        mask_t[:, 0:wm])
                                else:
                                    nc.gpsimd.affine_select(
                                        out=pt_[:, sl], in_=pt_[:, sl],
                                        compare_op=GE, fill=0.0, base=0,
                                        pattern=[[1, wm]],
                                        channel_multiplier=-1)
                        inflight.append((pt_, chs))
                    # flush ALL ctx matmuls at head end: the diagonal pairs'
                    # ctx then never head-of-line-blocks the in-order TM queue
                    # on a mask that waits on a backlogged exp (the
                    # TM->gpsimd->scalar->TM dependency cycle measured as
                    # 6-8us stalls at head boundaries)
                    for it in inflight:
                        ctx_mms(*it)
                    # pop the previous head's deferred norm only after this
                    # head's masks are all enqueued: the norm's gpsimd
                    # partition_broadcast otherwise head-of-line-blocks the
                    # mask queue while waiting on its ctx psum (measured
                    # 8us gpsimd stalls)
                    if pending:
                        pending.pop(0)()
                    pending.append(mk_norm(
                        h, j, m, r0, ctx_ps,
                        split=(j == NTB - 1 and h == HL - 1)))
                for g in main + reserve:
                    g()
                # partially-opened output groups (ci 0..2) keep the PE busy
                # while the last head's norm chain drains
                for g in pre_flush:
                    g()
                # flush deferred norms so output(j) can run during project(j+1)
                while pending:
                    pending.pop(0)()

            for g in project_groups(0, x_next, x8_next):
                g()
            # xpool bufs=2: x(2)/x(3) post TWO phases early (no WAR on the
            # previous block's readers) so proj fillers never wait DMAs;
            # x(1) stays just-in-time so it doesn't starve block-0's
            # xall/wo DMAs on the sync queue
            xs = {}
            for tb in range(NTB):
                ilq = []
                if tb == 0:
                    xs[1] = load_x(1)
                if tb + 2 < NTB:
                    xs[tb + 2] = load_x(tb + 2)
                if tb >= 1:
                    # output(tb-1) first: it has no dependence on the x(tb+1)
                    # DMAs still in flight, so early-attend fillers never
                    # head-of-line-block the TM queue on a DMA wait
                    ilq += output_groups(tb - 1)
                if tb + 1 < NTB:
                    ilq += project_groups(tb + 1, *xs[tb + 1])
                if tb == NTB - 1:
                    pre, post = output_last_split()
                    attend(tb, ilq, pre_flush=pre)
                    for g in post:
                        g()
                else:
                    attend(tb, ilq)

    nc.compile()
    return nc


def make_in_maps(x, Wq, Wk, Wv, Wo):
    import ml_dtypes
    bf = ml_dtypes.bfloat16
    f8 = ml_dtypes.float8_e4m3fn
    x = np.asarray(x, np.float32)
    Wq, Wk, Wv, Wo = (np.asarray(w, np.float32) for w in (Wq, Wk, Wv, Wo))
    CCH, MCH, NTB = C // P, OC // P, T_FULL // TB

    def packc(wT):  # [C, W] -> [P, CCH * W] chunk-major
        return np.ascontiguousarray(
            wT.reshape(-1, P, wT.shape[1]).transpose(1, 0, 2).reshape(P, -1))

    def packx(xT):  # [C, T] -> [P, NTB * CCH * TB] block-major then chunk
        return np.ascontiguousarray(
            xT.reshape(CCH, P, NTB, TB).transpose(1, 2, 0, 3).reshape(P, -1))

    in_maps = []
    for core in range(NCORES):
        b, g = divmod(core, GROUPS)
        sl = slice(g * OC, (g + 1) * OC)
        xTb = np.ascontiguousarray(x[b].T).astype(bf)
        wqTb = np.ascontiguousarray(Wq[sl, :].T).astype(bf)
        wkTb = np.ascontiguousarray(Wk[sl, :].T).astype(bf)
        in_maps.append({
            "xB": packx(xTb),
            "xB8": packx(xTb.astype(f8)),
            "wqB8": packc(wqTb.astype(f8)),
            "wkB8": packc(wkTb.astype(f8)),
            "wvB": packc(np.ascontiguousarray(Wv[sl, :].T).astype(bf)),
            "woB": packc(np.ascontiguousarray(Wo[:, sl].T).astype(bf)),
        })
    return in_maps


def _run(inputs, trace=False):
    from concourse.bass_utils import run_bass_kernel_spmd

    nc = build_program()
    in_maps = make_in_maps(
        inputs["x"], inputs["Wq"], inputs["Wk"], inputs["Wv"], inputs["Wo"])
    res = run_bass_kernel_spmd(nc, in_maps, core_ids=list(range(NCORES)), trace=trace)
    y = np.zeros((B, T_FULL, C), np.float32)
    for core in range(NCORES):
        y[core // GROUPS] += res.results[core]["yT"].T.astype(np.float32)
    return y, res


def kernel(**inputs):
    y, _ = _run(inputs)
    return y

